# revision 13
# baseline (speedup 1.0000x reference)
"""Soft-KNN NLL loss (ASKLoss) Trainium2 kernel.

Problem: x[1024,128] queries vs x_ref[50000,128] bank,
  score = -||x - xr||_2, probs = softmax over the 50000 refs,
  soft_nns = probs @ onehot(y_ref) + 1e-6, loss = -mean(log(soft_nns[b, y[b]])).

Strategy: data-parallel over the query batch across the 8 cores (128
queries/core, full 50000-ref bank streamed through each core in fp16).

Per core:
  - d2[b, n] = ||x_b||^2 + ||xr_n||^2 - 2<x_b, xr_n> built as:
      PE:  psum  = (-2 x^T)^T @ xr^T        (K=128 fp16 matmul)
      PE:  psum += ones^T @ xrnorm          (K=1 fp16 matmul, rank-1 row add)
      ACT: s = Sqrt(psum + ||x_b||^2)       (per-partition bias; PSUM->SBUF f16)
  - refs are class-sorted host-side into per-class slots (pad slots get
    xrnorm = 3e4 so exp(-sqrt(.)) == 0), so one ACT op per class:
      ACT: Exp(-s[class slot]) with accum_out -> per-class sum  [128, 10]
  - The Sqrt and Exp table sets are distinct, so the two phases are strictly
    ordered via a fence tile (zeros) that every Exp reads as its bias.

Host: concat the per-core [128, 10] class sums, compute the NLL in f64.
"""

import os

import numpy as np

import concourse.bass as bass
import concourse.mybir as mybir
import concourse.tile as tile
from concourse import bacc
from concourse.bass_utils import run_bass_kernel_spmd

B, N, D, C = 1024, 50000, 128, 10
N_CORES = 8
B_LOC = B // N_CORES           # 128 queries per core: one partition block

PAD_NORM = 30000.0             # pad slots: exp(-sqrt(3e4)) == 0
GROUP = 2048                   # ref columns per PSUM tile (4 banks)

F16 = mybir.dt.float16
F32 = mybir.dt.float32

LAST = {}                      # test harness introspection


def _build_module(caps):
    """Build the SPMD Bass module for per-class slot sizes `caps` (len C)."""
    caps = [int(c) for c in caps]
    offs = np.concatenate([[0], np.cumsum(caps)]).astype(int)
    n_pad = int(offs[-1])
    max_cap = max(caps)

    nc = bacc.Bacc(
        "TRN2",
        target_bir_lowering=False,
        debug=False,
        enable_asserts=True,
        num_devices=N_CORES,
    )

    xT2_d = nc.dram_tensor("xT2", [D, B_LOC], F16, kind="ExternalInput")
    xrT_d = nc.dram_tensor("xrT", [D, n_pad], F16, kind="ExternalInput")
    xrn_d = nc.dram_tensor("xrnorm", [1, n_pad], F16, kind="ExternalInput")
    xn_d = nc.dram_tensor("xnorm", [128, 1], F32, kind="ExternalInput")
    ones_d = nc.dram_tensor("ones", [1, 128], F16, kind="ExternalInput")
    cls_d = nc.dram_tensor("cls", [128, C], F32, kind="ExternalOutput")

    # group layout: two small lead-in groups so the first Sqrts (and the ACT
    # pipeline) start early, then full 2048-wide groups
    bounds = [0]
    for lead in (512, 1024):
        if bounds[-1] + lead < n_pad:
            bounds.append(bounds[-1] + lead)
    while bounds[-1] < n_pad:
        bounds.append(min(bounds[-1] + GROUP, n_pad))
    groups = list(zip(bounds[:-1], bounds[1:]))

    with tile.TileContext(nc) as tc:
        with (
            tc.tile_pool(name="const", bufs=1) as const_pool,
            tc.tile_pool(name="xr", bufs=3) as xr_pool,
            tc.tile_pool(name="xrn", bufs=3) as xrn_pool,
            tc.tile_pool(name="sbig", bufs=1) as s_pool,
            tc.tile_pool(name="scr", bufs=2) as scr_pool,
            tc.tile_pool(name="psum", bufs=2, space="PSUM") as psum_pool,
        ):
            xT2 = const_pool.tile([D, B_LOC], F16)
            xn = const_pool.tile([128, 1], F32)
            ones = const_pool.tile([1, 128], F16)
            cls = const_pool.tile([128, C], F32)

            # Warm-up: force the Sqrt table set to load at t~0 (a dependency-
            # free Sqrt on a memset tile) instead of right before the first
            # real Sqrt; the table DMA then overlaps the input DMAs/matmuls.
            warm = const_pool.tile([128, 1], F32)
            nc.gpsimd.memset(warm[:], 1.0)
            nc.scalar.activation(
                warm[:], warm[:], mybir.ActivationFunctionType.Sqrt
            )

            # latency-critical transfers on the HWDGE (sync) queue, smallest
            # first; xn (only needed by the first Sqrt) rides the gpsimd queue
            nc.sync.dma_start(ones[:], ones_d.ap())
            nc.sync.dma_start(xT2[:], xT2_d.ap())
            nc.gpsimd.dma_start(xn[:], xn_d.ap())

            # fp16 s-values for every (query, ref) pair on this core
            s_sb = s_pool.tile([128, n_pad], F16)

            # ---- Phase 1: stream bank, matmuls + Sqrt into s_sb ----
            for g0, g1 in groups:
                w = g1 - g0
                xr_t = xr_pool.tile([D, GROUP], F16, tag="xr")
                xrn_t = xrn_pool.tile([1, GROUP], F16, tag="xrn")
                nc.sync.dma_start(xr_t[:, :w], xrT_d.ap()[:, g0 : g0 + w])
                nc.sync.dma_start(xrn_t[:, :w], xrn_d.ap()[:, g0 : g0 + w])

                d2 = psum_pool.tile([128, GROUP], F32, tag="d2")
                # mains first (shared stationary xT2), then rank-1 bias rows
                for j0 in range(0, w, 512):
                    jw = min(512, w - j0)
                    nc.tensor.matmul(
                        d2[:, j0 : j0 + jw],
                        xT2[:],
                        xr_t[:, j0 : j0 + jw],
                        start=True,
                        stop=False,
                    )
                for j0 in range(0, w, 512):
                    jw = min(512, w - j0)
                    nc.tensor.matmul(
                        d2[:, j0 : j0 + jw],
                        ones[:],
                        xrn_t[:, j0 : j0 + jw],
                        start=False,
                        stop=True,
                    )
                nc.scalar.activation(
                    s_sb[:, g0 : g0 + w],
                    d2[:, :w],
                    mybir.ActivationFunctionType.Sqrt,
                    bias=xn[:, 0:1],
                    scale=1.0,
                )

            # ---- Phase fence: every Exp reads (as bias) a zeros tile derived
            # from the LAST Sqrt output, so the scheduler cannot interleave the
            # Exp phase into the Sqrt phase (ACT table-set thrash, ~2.7us/switch).
            fence0 = const_pool.tile([128, 1], F32)
            nc.vector.tensor_scalar_mul(
                fence0[:], s_sb[:, n_pad - 1 : n_pad], 0.0
            )

            # ---- Phase 2: Exp with accumulate -> per-class sums ----
            for k in range(C):
                e_scr = scr_pool.tile([128, max_cap], F16, tag="escr")
                nc.scalar.activation(
                    e_scr[:, : caps[k]],
                    s_sb[:, offs[k] : offs[k + 1]],
                    mybir.ActivationFunctionType.Exp,
                    bias=fence0[:, 0:1],
                    scale=-1.0,
                    accum_out=cls[:, k : k + 1],
                )

            nc.sync.dma_start(cls_d.ap(), cls[:])

    nc.compile()
    return nc, {
        "xT2": xT2_d.name,
        "xrT": xrT_d.name,
        "xrnorm": xrn_d.name,
        "xnorm": xn_d.name,
        "ones": ones_d.name,
        "cls": cls_d.name,
    }


def _prepare_inputs(x, x_ref, y_ref, caps):
    """Sorted/padded bank (shared) + per-core query blocks."""
    offs = np.concatenate([[0], np.cumsum(caps)]).astype(int)
    n_pad = int(offs[-1])

    x = np.asarray(x, dtype=np.float32)
    x_ref = np.asarray(x_ref, dtype=np.float32)
    y_ref = np.asarray(y_ref).astype(np.int64)

    xnorm = (x.astype(np.float64) ** 2).sum(axis=1).astype(np.float32)  # [B]
    xrnorm = (x_ref.astype(np.float64) ** 2).sum(axis=1).astype(np.float32)  # [N]

    order = np.argsort(y_ref, kind="stable")
    counts = np.bincount(y_ref, minlength=C)
    xrT_pad = np.zeros((D, n_pad), dtype=np.float16)
    xrn_pad = np.full((1, n_pad), PAD_NORM, dtype=np.float16)
    pos = 0
    for k in range(C):
        cnt = int(counts[k])
        assert cnt <= caps[k], (k, cnt, caps[k])
        idx = order[pos : pos + cnt]
        pos += cnt
        xrT_pad[:, offs[k] : offs[k] + cnt] = x_ref[idx].T.astype(np.float16)
        xrn_pad[0, offs[k] : offs[k] + cnt] = xrnorm[idx].astype(np.float16)

    blocks = []
    for i in range(N_CORES):
        sl = slice(i * B_LOC, (i + 1) * B_LOC)
        xT2 = (-2.0 * x[sl].T).astype(np.float16)  # [D, B_LOC]
        xn_t = xnorm[sl].reshape(B_LOC, 1).copy()  # [128, 1]
        blocks.append((xT2, xn_t))

    ones = np.ones((1, 128), dtype=np.float16)
    return xrT_pad, xrn_pad, blocks, ones


def kernel(x, x_ref, y, y_ref):
    x = np.asarray(x)
    x_ref = np.asarray(x_ref)
    y = np.asarray(y).astype(np.int64)
    y_ref_i = np.asarray(y_ref).astype(np.int64)

    counts = np.bincount(y_ref_i, minlength=C)
    caps = [max(16, ((int(c) + 15) // 16) * 16) for c in counts]

    nc, names = _build_module(caps)
    xrT_pad, xrn_pad, blocks, ones = _prepare_inputs(x, x_ref, y_ref_i, caps)

    in_maps = []
    for core in range(N_CORES):
        xT2, xn_t = blocks[core]
        in_maps.append(
            {
                names["xT2"]: xT2,
                names["xrT"]: xrT_pad,
                names["xrnorm"]: xrn_pad,
                names["xnorm"]: xn_t,
                names["ones"]: ones,
            }
        )

    trace = bool(int(os.environ.get("KERNEL_TRACE", "0")))
    res = run_bass_kernel_spmd(
        nc, in_maps, core_ids=list(range(N_CORES)), trace=trace
    )
    LAST["exec_time_ns"] = res.exec_time_ns
    LAST["results"] = res
    LAST["module"] = nc

    # ---- host combine: concat per-core class sums, then NLL ----
    cs = np.concatenate(
        [np.asarray(res.results[core][names["cls"]], dtype=np.float64)
         for core in range(N_CORES)],
        axis=0,
    )  # [B, C]

    total = cs.sum(axis=1, keepdims=True)
    soft = cs / total + 1e-6
    loss = -np.mean(np.log(soft[np.arange(B), y]))
    return np.float32(loss)


# revision 31
# speedup vs baseline: 1.0220x; 1.0220x over previous
"""Soft-KNN NLL loss (ASKLoss) Trainium2 kernel.

Problem: x[1024,128] queries vs x_ref[50000,128] bank,
  score = -||x - xr||_2, probs = softmax over the 50000 refs,
  soft_nns = probs @ onehot(y_ref) + 1e-6, loss = -mean(log(soft_nns[b, y[b]])).

Strategy: data-parallel over the query batch across the 8 cores (128
queries/core, full 50000-ref bank streamed through each core in fp16).

Per core:
  - d2[b, n] = ||x_b||^2 + ||xr_n||^2 - 2<x_b, xr_n> built as:
      PE:  psum  = (-2 x^T)^T @ xr^T        (K=128 fp16 matmul)
      PE:  psum += ones^T @ xrnorm          (K=1 fp16 matmul, rank-1 row add)
      ACT: s = Sqrt(psum + ||x_b||^2)       (per-partition bias; PSUM->SBUF f16)
  - refs are class-sorted host-side into per-class slots (pad slots get
    xrnorm = 3e4 so exp(-sqrt(.)) == 0), so one ACT op per class:
      ACT: Exp(-s[class slot]) with accum_out -> per-class sum  [128, 10]
  - The Sqrt and Exp table sets are distinct, so the two phases are strictly
    ordered via a fence tile (zeros) that every Exp reads as its bias.

Host: concat the per-core [128, 10] class sums, compute the NLL in f64.
"""

import os

import numpy as np

import concourse.bass as bass
import concourse.mybir as mybir
import concourse.tile as tile
from concourse import bacc
from concourse.bass_utils import run_bass_kernel_spmd

B, N, D, C = 1024, 50000, 128, 10
N_CORES = 8
B_LOC = B // N_CORES           # 128 queries per core: one partition block

PAD_NORM = 30000.0             # pad slots: exp(-sqrt(3e4)) == 0
GROUP = 2048                   # ref columns per PSUM tile (4 banks)

F16 = mybir.dt.float16
F32 = mybir.dt.float32

LAST = {}                      # test harness introspection
_MODULE_CACHE = {}             # caps tuple -> (nc, names); reuse across calls


def _build_module(caps):
    """Build the SPMD Bass module for per-class slot sizes `caps` (len C)."""
    caps = [int(c) for c in caps]
    offs = np.concatenate([[0], np.cumsum(caps)]).astype(int)
    n_pad = int(offs[-1])
    max_cap = max(caps)

    nc = bacc.Bacc(
        "TRN2",
        target_bir_lowering=False,
        debug=False,
        enable_asserts=True,
        num_devices=N_CORES,
    )

    xT2_d = nc.dram_tensor("xT2", [D, B_LOC], F16, kind="ExternalInput")
    xrT_d = nc.dram_tensor("xrT", [D, n_pad], F16, kind="ExternalInput")
    xrn_d = nc.dram_tensor("xrnorm", [1, n_pad], F16, kind="ExternalInput")
    xn_d = nc.dram_tensor("xnorm", [128, 1], F32, kind="ExternalInput")
    cls_d = nc.dram_tensor("cls", [128, C], F32, kind="ExternalOutput")

    # uniform 2048-wide groups (ragged tail); DMA dispatch (~650ns/inst on the
    # SP sequencer) dominates the pipeline fill, so fewer/larger transfers win
    bounds = list(range(0, n_pad, GROUP)) + [n_pad]
    groups = list(zip(bounds[:-1], bounds[1:]))

    with tile.TileContext(nc) as tc:
        with (
            tc.tile_pool(name="const", bufs=1) as const_pool,
            tc.tile_pool(name="xr", bufs=3) as xr_pool,
            tc.tile_pool(name="xrn", bufs=3) as xrn_pool,
            tc.tile_pool(name="sbig", bufs=1) as s_pool,
            tc.tile_pool(name="scr", bufs=2) as scr_pool,
            tc.tile_pool(name="psum", bufs=2, space="PSUM") as psum_pool,
        ):
            xT2 = const_pool.tile([D, B_LOC], F16)
            xn = const_pool.tile([128, 1], F32)
            ones = const_pool.tile([1, 128], F16)
            cls = const_pool.tile([128, C], F32)

            # ones is constant: memset (Pool engine, t~0) instead of a DMA —
            # every DMA dispatch in the startup window costs ~650ns of SP.SEQ
            nc.gpsimd.memset(ones[:], 1.0)

            # Warm-up: force the Sqrt table set to load at t~0 (a dependency-
            # free Sqrt on a memset tile) instead of right before the first
            # real Sqrt; the table DMA then overlaps the input DMAs/matmuls.
            warm = const_pool.tile([128, 1], F32)
            nc.gpsimd.memset(warm[:], 1.0)
            nc.scalar.activation(
                warm[:], warm[:], mybir.ActivationFunctionType.Sqrt
            )

            # fp16 s-values for every (query, ref) pair on this core
            s_sb = s_pool.tile([128, n_pad], F16)

            # ---- Phase 1: stream bank, matmuls + Sqrt into s_sb ----
            # per-group stream DMAs issue first (SP dispatch order == program
            # order); xn (only gates the first Sqrt) rides the gpsimd queue
            for gi, (g0, g1) in enumerate(groups):
                w = g1 - g0
                xrn_t = xrn_pool.tile([1, GROUP], F16, tag="xrn")
                nc.sync.dma_start(xrn_t[:, :w], xrn_d.ap()[:, g0 : g0 + w])
                xrn_base = g0
                xr_t = xr_pool.tile([D, GROUP], F16, tag="xr")
                nc.sync.dma_start(xr_t[:, :w], xrT_d.ap()[:, g0 : g0 + w])
                if gi == 0:
                    nc.sync.dma_start(xT2[:], xT2_d.ap())
                    nc.gpsimd.dma_start(xn[:], xn_d.ap())

                d2 = psum_pool.tile([128, GROUP], F32, tag="d2")
                # rank-1 bias rows first (xrnorm lands well before the wide xr
                # tile and ones needs no DMA), then the mains
                for j0 in range(0, w, 512):
                    jw = min(512, w - j0)
                    nc.tensor.matmul(
                        d2[:, j0 : j0 + jw],
                        ones[:],
                        xrn_t[:, g0 - xrn_base + j0 : g0 - xrn_base + j0 + jw],
                        start=True,
                        stop=False,
                    )
                for j0 in range(0, w, 512):
                    jw = min(512, w - j0)
                    nc.tensor.matmul(
                        d2[:, j0 : j0 + jw],
                        xT2[:],
                        xr_t[:, j0 : j0 + jw],
                        start=False,
                        stop=True,
                    )
                nc.scalar.activation(
                    s_sb[:, g0 : g0 + w],
                    d2[:, :w],
                    mybir.ActivationFunctionType.Sqrt,
                    bias=xn[:, 0:1],
                    scale=1.0,
                )

            # ---- Phase fence: every Exp reads (as bias) a zeros tile derived
            # from the LAST Sqrt output, so the scheduler cannot interleave the
            # Exp phase into the Sqrt phase (ACT table-set thrash, ~2.7us/switch).
            fence0 = const_pool.tile([128, 1], F32)
            nc.vector.tensor_scalar_mul(
                fence0[:], s_sb[:, n_pad - 1 : n_pad], 0.0
            )

            # ---- Phase 2: Exp with accumulate -> per-class sums ----
            order_k = sorted(range(C), key=lambda k: -caps[k])
            for k in order_k:
                e_scr = scr_pool.tile([128, max_cap], F16, tag="escr")
                nc.scalar.activation(
                    e_scr[:, : caps[k]],
                    s_sb[:, offs[k] : offs[k + 1]],
                    mybir.ActivationFunctionType.Exp,
                    bias=fence0[:, 0:1],
                    scale=-1.0,
                    accum_out=cls[:, k : k + 1],
                )
            nc.sync.dma_start(cls_d.ap(), cls[:])

    nc.compile()
    return nc, {
        "xT2": xT2_d.name,
        "xrT": xrT_d.name,
        "xrnorm": xrn_d.name,
        "xnorm": xn_d.name,
        "cls": cls_d.name,
    }


def _prepare_inputs(x, x_ref, y_ref, caps):
    """Sorted/padded bank (shared) + per-core query blocks."""
    offs = np.concatenate([[0], np.cumsum(caps)]).astype(int)
    n_pad = int(offs[-1])

    x = np.asarray(x, dtype=np.float32)
    x_ref = np.asarray(x_ref, dtype=np.float32)
    y_ref = np.asarray(y_ref).astype(np.int64)

    xnorm = (x.astype(np.float64) ** 2).sum(axis=1).astype(np.float32)  # [B]
    xrnorm = (x_ref.astype(np.float64) ** 2).sum(axis=1).astype(np.float32)  # [N]

    order = np.argsort(y_ref, kind="stable")
    counts = np.bincount(y_ref, minlength=C)
    xrT_pad = np.zeros((D, n_pad), dtype=np.float16)
    xrn_pad = np.full((1, n_pad), PAD_NORM, dtype=np.float16)
    pos = 0
    for k in range(C):
        cnt = int(counts[k])
        assert cnt <= caps[k], (k, cnt, caps[k])
        idx = order[pos : pos + cnt]
        pos += cnt
        xrT_pad[:, offs[k] : offs[k] + cnt] = x_ref[idx].T.astype(np.float16)
        xrn_pad[0, offs[k] : offs[k] + cnt] = xrnorm[idx].astype(np.float16)

    blocks = []
    for i in range(N_CORES):
        sl = slice(i * B_LOC, (i + 1) * B_LOC)
        xT2 = (-2.0 * x[sl].T).astype(np.float16)  # [D, B_LOC]
        xn_t = xnorm[sl].reshape(B_LOC, 1).copy()  # [128, 1]
        blocks.append((xT2, xn_t))

    return xrT_pad, xrn_pad, blocks


def kernel(x, x_ref, y, y_ref):
    x = np.asarray(x)
    x_ref = np.asarray(x_ref)
    y = np.asarray(y).astype(np.int64)
    y_ref_i = np.asarray(y_ref).astype(np.int64)

    counts = np.bincount(y_ref_i, minlength=C)
    caps = [max(16, ((int(c) + 15) // 16) * 16) for c in counts]

    key = tuple(caps)
    if key not in _MODULE_CACHE:
        _MODULE_CACHE[key] = _build_module(caps)
    nc, names = _MODULE_CACHE[key]
    xrT_pad, xrn_pad, blocks = _prepare_inputs(x, x_ref, y_ref_i, caps)

    in_maps = []
    for core in range(N_CORES):
        xT2, xn_t = blocks[core]
        in_maps.append(
            {
                names["xT2"]: xT2,
                names["xrT"]: xrT_pad,
                names["xrnorm"]: xrn_pad,
                names["xnorm"]: xn_t,
            }
        )

    trace = bool(int(os.environ.get("KERNEL_TRACE", "0")))
    res = run_bass_kernel_spmd(
        nc, in_maps, core_ids=list(range(N_CORES)), trace=trace
    )
    LAST["exec_time_ns"] = res.exec_time_ns
    LAST["results"] = res
    LAST["module"] = nc

    # ---- host combine: concat per-core class sums, then NLL ----
    cs = np.concatenate(
        [np.asarray(res.results[core][names["cls"]], dtype=np.float64)
         for core in range(N_CORES)],
        axis=0,
    )  # [B, C]

    total = cs.sum(axis=1, keepdims=True)
    soft = cs / total + 1e-6
    loss = -np.mean(np.log(soft[np.arange(B), y]))
    return np.asarray(loss, dtype=np.float32)
